# revision 25
# baseline (speedup 1.0000x reference)
"""Two-layer GAT on 8 Trainium2 NeuronCores (Bass/Tile), bf16 gather path.

Host (numpy): append self-loops, degree-sort nodes (desc), pad node count to
VPAD (multiple of 8*128) and assign sorted nodes round-robin at 128-node
block granularity to the 8 cores (sorted-rank s -> block g=s//128,
lane=s%128 -> core c=g%8, local block j=g//8, table row = c*PC+j*128+lane).
Per block-rank j the chunk schedule is shared by all cores (SPMD: one
program, per-core tensor data).  Each dst node's edges occupy "slots"; a
chunk is slot k of all 128 lanes of a block.  Edge slots are split into a
lo-region (src table row < VPAD/2) and hi-region so every chunk's gather
indices fit in int16 after rebasing (dma_gather is int16-indexed).

The gather table holds ONLY h (+bias) as 128 bf16 = 256B rows (the dma
gather minimum).  Per-edge alpha_src is recomputed on-chip from the
gathered h row via a DVE multiply + free-dim reduce against a replicated
a_src constant; alpha_dst stays a per-lane [128,1] activation bias
(lane-aligned slots).  Softmax denominators come from a tensor_reduce over
the per-block z tile instead of extra V columns, so the segment-sum matmul
V is a pure 128-col bf16 tile accumulated into PSUM with an identity lhsT.
x, W, the table, and V are all bf16; PSUM accumulation fp32.
"""

import numpy as np
import ml_dtypes

BF16 = ml_dtypes.bfloat16

NCORES = 8
F_IN = 128
HID = 64
HEADS = 2
OUT = 64
NEG_SLOPE = 0.2

GBATCH = 64  # max chunks per dma_gather
KMAX = 64    # z-tile slot capacity (assert K.max() <= KMAX)
PIECES = 4   # AllGather pieces (table row space is piece-major)

TRACE = False
_cache = {}


def _build_schedule(edge_index, n_nodes):
    ei = np.asarray(edge_index).astype(np.int64)
    src = np.concatenate([ei[0], np.arange(n_nodes, dtype=np.int64)])
    dst = np.concatenate([ei[1], np.arange(n_nodes, dtype=np.int64)])
    deg = np.bincount(dst, minlength=n_nodes)

    stripe = NCORES * 128
    vpad = ((n_nodes + stripe - 1) // stripe) * stripe
    pc = vpad // NCORES
    nb = pc // 128
    half = vpad // 2
    assert half <= 32768

    # AllGather piece boundaries (in blocks); the gather-table row space is
    # piece-major (piece, core, block-within-piece, lane) so each piece's
    # collective output is one contiguous slice of h_full.
    cuts = sorted(set(min(nb, ((nb * (i + 1)) + PIECES - 1) // PIECES)
                      for i in range(PIECES)))
    starts = [0] + cuts[:-1]
    piece_of_j = np.zeros(nb, np.int64)
    tbase_of_j = np.zeros(nb, np.int64)
    cstr_of_j = np.zeros(nb, np.int64)
    joff_of_j = np.zeros(nb, np.int64)
    base = 0
    for p, (j0, j1) in enumerate(zip(starts, cuts)):
        bp = j1 - j0
        for j in range(j0, j1):
            piece_of_j[j] = p
            tbase_of_j[j] = base
            cstr_of_j[j] = bp * 128
            joff_of_j[j] = (j - j0) * 128
        base += NCORES * bp * 128
    assert base == vpad

    degp = np.zeros(vpad, np.int64)
    degp[:n_nodes] = deg
    order = np.argsort(-degp, kind="stable")
    rank = np.empty(vpad, np.int64)
    rank[order] = np.arange(vpad)

    s = np.arange(vpad)
    g = s // 128
    lane_r = s % 128
    c_r = g % NCORES
    j_r = g // NCORES
    local_of_rank = c_r * pc + j_r * 128 + lane_r
    trow_of_rank = (tbase_of_j[j_r] + c_r * cstr_of_j[j_r]
                    + joff_of_j[j_r] + lane_r)
    nrank = rank[:n_nodes]
    row_of_node = local_of_rank[nrank]       # for x placement / output
    trow_of_node = trow_of_rank[nrank]       # for gather indices
    c_of_node = c_r[nrank]
    j_of_node = j_r[nrank]
    lane_of_node = lane_r[nrank]

    e_srcrow = trow_of_node[src]
    is_hi = e_srcrow >= half
    e_c = c_of_node[dst]
    e_j = j_of_node[dst]
    e_lane = lane_of_node[dst]
    e_dkey = (e_c * pc // 128 + e_j) * 128 + e_lane  # dense (c, j, lane) id

    # per-(dst, lo/hi) counts
    nkeys = NCORES * nb * 128
    lo_cnt = np.bincount(e_dkey[~is_hi], minlength=nkeys)
    hi_cnt = np.bincount(e_dkey[is_hi], minlength=nkeys)

    # per-block-rank shared chunk counts: max over the 8 cores' j-th blocks
    jj = (np.arange(nkeys) // 128) % nb
    K_lo = np.zeros(nb, np.int64)
    K_hi = np.zeros(nb, np.int64)
    np.maximum.at(K_lo, jj, lo_cnt)
    np.maximum.at(K_hi, jj, hi_cnt)
    K = K_lo + K_hi
    bump = K == 0
    K_lo[bump] += 1
    K[bump] += 1
    nch = int(K.sum())
    chunk_base = np.concatenate([[0], np.cumsum(K)])[:-1]

    # slot assignment: edges of a dst grouped, lo first then hi
    key = e_dkey * 2 + is_hi
    ord_e = np.argsort(key, kind="stable")
    ds = e_dkey[ord_e]
    hs = is_hi[ord_e]
    first = np.r_[True, ds[1:] != ds[:-1]]
    grp_start = np.flatnonzero(first)
    grp_id = np.cumsum(first) - 1
    slot = np.arange(ds.shape[0]) - grp_start[grp_id]
    c = ds // (nb * 128)
    j = (ds // 128) % nb
    ln = ds % 128
    pos = chunk_base[j] + np.where(hs, K_lo[j] + slot - lo_cnt[ds], slot)
    assert (pos >= chunk_base[j]).all() and (pos < chunk_base[j] + K[j]).all()

    idx_val = np.where(hs, e_srcrow[ord_e] - half, e_srcrow[ord_e])
    idx_stream = np.zeros((NCORES, 128, nch), np.int16)
    mask_stream = np.zeros((NCORES, 128, nch), BF16)
    idx_stream[c, ln, pos] = idx_val.astype(np.int16)
    mask_stream[c, ln, pos] = 1.0

    # wrapped int16 layout for dma_gather: chunk k -> columns 8k:8k+8 of
    # [128, 8*nch]; within a chunk the 128 lane-indices are wrapped as
    # flat[i] -> [i % 16, i // 16] and replicated over the 8 16-partition
    # groups.
    iw = idx_stream.transpose(0, 2, 1).reshape(NCORES, nch, 8, 16)
    iw = iw.transpose(0, 3, 1, 2).reshape(NCORES, 16, nch * 8)
    idx_wrapped = np.tile(iw, (1, 8, 1))

    return dict(vpad=vpad, pc=pc, nb=nb, half=half, K=K, K_lo=K_lo, K_hi=K_hi,
                nch=nch, chunk_base=chunk_base, row_of_node=row_of_node,
                cuts=tuple(cuts),
                idx_wrapped=np.ascontiguousarray(idx_wrapped),
                mask_stream=np.ascontiguousarray(mask_stream))


def _build_program(vpad, pc, nb, half, K, K_lo, K_hi, nch, chunk_base, cuts):
    import concourse.bacc as bacc
    import concourse.bass as bass
    import concourse.mybir as mybir
    import concourse.tile as tile
    from concourse.masks import make_identity

    F32 = mybir.dt.float32
    BF = mybir.dt.bfloat16
    I16 = mybir.dt.int16
    ACTF = mybir.ActivationFunctionType
    ALU = mybir.AluOpType
    AXL = mybir.AxisListType

    assert int(K.max()) <= KMAX

    nc = bacc.Bacc("TRN2", target_bir_lowering=False, debug=False,
                   num_devices=NCORES, num_swdge_queues=4)
    qn = [0]

    def next_q():
        q = qn[0]
        qn[0] = (q + 1) % 4
        return q

    xt_d = nc.dram_tensor("xt", [128, pc], BF, kind="ExternalInput")
    idx_d = nc.dram_tensor("idx", [128, nch * 8], I16, kind="ExternalInput")
    msk_d = nc.dram_tensor("msk", [128, nch], BF, kind="ExternalInput")
    w1_d = nc.dram_tensor("w1aug", [128, 130], BF, kind="ExternalInput")
    w2_d = nc.dram_tensor("w2aug", [128, 66], BF, kind="ExternalInput")
    b1_d = nc.dram_tensor("b1rep", [128, 128], F32, kind="ExternalInput")
    b2_d = nc.dram_tensor("b2rep", [128, 64], F32, kind="ExternalInput")
    as_d = nc.dram_tensor("asrcrep", [128, 128], BF, kind="ExternalInput")
    out_d = nc.dram_tensor("out", [pc, OUT], F32, kind="ExternalOutput")

    with tile.TileContext(nc) as tc:
        with (
            tc.tile_pool(name="const", bufs=1) as cp,
            tc.tile_pool(name="dram", bufs=1, space="DRAM") as dp,
            tc.tile_pool(name="hrow", bufs=3) as hp,
            tc.tile_pool(name="psh", bufs=2, space="PSUM") as psh,
            tc.tile_pool(name="g", bufs=4) as gp,
            tc.tile_pool(name="sc", bufs=2) as sp,
            tc.tile_pool(name="v", bufs=2) as vp,
            tc.tile_pool(name="z", bufs=2) as zp,
            tc.tile_pool(name="wz", bufs=4) as wp,
            tc.tile_pool(name="psa", bufs=2, space="PSUM") as psa,
            tc.tile_pool(name="pst", bufs=1, space="PSUM") as pst,
            tc.tile_pool(name="epi", bufs=3) as ep,
        ):
            ident = cp.tile([128, 128], BF)
            make_identity(nc, ident[:])
            w1_sb = cp.tile([128, 130], BF)
            w2_sb = cp.tile([128, 66], BF)
            b1_sb = cp.tile([128, 128], F32)
            b2_sb = cp.tile([128, 64], F32)
            as_sb = cp.tile([128, 128], BF)
            idx_t = cp.tile([128, nch * 8], I16)
            msk_t = cp.tile([128, nch], BF)
            xt_all = cp.tile([128, pc], BF)
            adst1 = cp.tile([128, 2 * nb], F32)
            adst2 = cp.tile([128, nb], F32)
            for t, d in ((w1_sb, w1_d), (w2_sb, w2_d), (b1_sb, b1_d),
                         (b2_sb, b2_d), (as_sb, as_d), (idx_t, idx_d),
                         (msk_t, msk_d), (xt_all, xt_d)):
                nc.sync.dma_start(out=t[:], in_=d[:])

            h_loc = dp.tile([pc, 128], BF)
            h2_loc = dp.tile([pc, 128], BF)
            h_full = dp.tile([vpad, 128], BF)
            h2_full = dp.tile([vpad, 128], BF)

            # piece p = blocks [starts[p], cuts[p]); its table rows are the
            # contiguous [tb, tb + 8*bp*128) slice (piece-major row space)
            starts = [0] + list(cuts[:-1])
            tbases = []
            tb = 0
            for j0, j1 in zip(starts, cuts):
                tbases.append(tb)
                tb += NCORES * (j1 - j0) * 128
            assert tb == vpad

            def ag_piece(loc, full, p):
                j0, j1 = starts[p], cuts[p]
                nc.gpsimd.collective_compute(
                    "AllGather", mybir.AluOpType.bypass,
                    replica_groups=[list(range(NCORES))],
                    ins=[loc[j0 * 128:j1 * 128, :]],
                    outs=[full[tbases[p]:tbases[p]
                               + NCORES * (j1 - j0) * 128, :]],
                )

            # ---- Phase 1: L1 h-phase (AllGather in overlapped pieces) ----
            for j in range(nb):
                ps = psh.tile([128, 130], F32, tag="psh")
                nc.tensor.matmul(ps[:], lhsT=xt_all[:, j * 128:(j + 1) * 128],
                                 rhs=w1_sb[:], start=True, stop=True)
                hrow = hp.tile([128, 128], BF, tag="hrow")
                nc.vector.tensor_tensor(out=hrow[:], in0=ps[:, 0:128],
                                        in1=b1_sb[:], op=ALU.add)
                nc.vector.tensor_copy(adst1[:, j * 2:(j + 1) * 2],
                                      ps[:, 128:130])
                nc.sync.dma_start(out=h_loc[j * 128:(j + 1) * 128, :],
                                  in_=hrow[:])
                if j + 1 in cuts:
                    ag_piece(h_loc, h_full, cuts.index(j + 1))

            def batches_of(j):
                out = []
                done = 0
                for rl in (int(K_lo[j]), int(K_hi[j])):
                    r0 = done
                    while done < r0 + rl:
                        gl = min(GBATCH, r0 + rl - done)
                        out.append((done, gl, done >= int(K_lo[j])))
                        done += gl
                return out

            def agg1(j):
                kj = int(K[j])
                cb = int(chunk_base[j])
                psum = psa.tile([128, 130], F32, tag="ps")
                z = zp.tile([128, 2, KMAX], BF, tag="z1")
                az = zp.tile([128, 2, KMAX], F32, tag="az")
                for (b0, gl, in_hi) in batches_of(j):
                    k0 = cb + b0
                    gt = gp.tile([128, GBATCH * 128], BF, tag="g")
                    tab_ap = h_full[half:vpad, :] if in_hi else h_full[0:half, :]
                    nc.gpsimd.dma_gather(
                        gt[:, 0:gl * 128].rearrange("p (k c) -> p k c", c=128),
                        tab_ap,
                        idx_t[:, k0 * 8:(k0 + gl) * 8],
                        gl * 128, gl * 128, 128,
                        single_packet=False, queue_num=next_q(),
                    )
                    gv = gt[:, 0:gl * 128].rearrange("p (k c) -> p k c", c=128)
                    # alpha_src per edge: dot(h_row, a_src) over each head's
                    # 64 cols, both heads in one mult + one reduce
                    scr = sp.tile([128, GBATCH * 128], BF, tag="s", bufs=1)
                    sv = scr[:, 0:gl * 128].rearrange(
                        "p (k h c) -> p k h c", h=2, c=HID)
                    nc.vector.tensor_tensor(
                        out=sv, in0=gv.rearrange("p k (h c) -> p k h c", c=HID),
                        in1=as_sb[:].rearrange("p (h c) -> p h c", c=HID)
                            .unsqueeze(1).broadcast_to([128, gl, 2, HID]),
                        op=ALU.mult)
                    azr = az[:, :, :].rearrange("p h k -> p k h")
                    nc.vector.tensor_reduce(
                        out=azr[:, b0:b0 + gl, :], in_=sv,
                        axis=AXL.X, op=ALU.add)
                    for h in range(HEADS):
                        nc.scalar.activation(
                            az[:, h, b0:b0 + gl], az[:, h, b0:b0 + gl],
                            ACTF.Prelu, bias=adst1[:, 2 * j + h:2 * j + h + 1],
                            alpha=NEG_SLOPE)
                        nc.scalar.activation(
                            z[:, h, b0:b0 + gl], az[:, h, b0:b0 + gl],
                            ACTF.Exp)
                    nc.vector.tensor_tensor(
                        out=z[:, :, b0:b0 + gl], in0=z[:, :, b0:b0 + gl],
                        in1=msk_t[:, k0:k0 + gl].unsqueeze(1)
                            .broadcast_to([128, 2, gl]),
                        op=ALU.mult)
                    vt = vp.tile([128, GBATCH * 130], BF, tag="v")
                    vv = vt[:, 0:gl * 130].rearrange("p (k c) -> p k c", c=130)
                    zr = z[:, :, b0:b0 + gl].rearrange("p h k -> p k h")
                    nc.vector.tensor_tensor(
                        out=vv[:, :, 0:128].rearrange(
                            "p k (h c) -> p k h c", c=HID),
                        in0=gv.rearrange("p k (h c) -> p k h c", c=HID),
                        in1=zr.unsqueeze(3).broadcast_to([128, gl, 2, HID]),
                        op=ALU.mult)
                    nc.vector.tensor_copy(vv[:, :, 128:130], zr)
                    for k in range(gl):
                        nc.tensor.matmul(
                            psum[:], lhsT=ident[:],
                            rhs=vt[:, k * 130:(k + 1) * 130],
                            start=(b0 + k == 0), stop=(b0 + k == kj - 1))

                # +1e-30 keeps all-padding lanes finite (0 -> 1e30 -> 0*1e30=0)
                dsafe = wp.tile([128, 2], F32, tag="dsafe")
                nc.vector.tensor_scalar_add(dsafe[:], psum[:, 128:130], 1e-30)
                rden = wp.tile([128, 2], F32, tag="rden")
                nc.vector.reciprocal(rden[:], dsafe[:])

                h2pre = ep.tile([128, 128], BF, tag="h2pre")
                for h in range(HEADS):
                    nc.vector.tensor_scalar(
                        out=h2pre[:, h * HID:(h + 1) * HID],
                        in0=psum[:, h * HID:(h + 1) * HID],
                        scalar1=rden[:, h:h + 1], scalar2=0.0,
                        op0=ALU.mult, op1=ALU.max)
                tps = pst.tile([128, 128], BF, tag="tps")
                nc.tensor.transpose(out=tps[:], in_=h2pre[:], identity=ident[:])
                h2t = ep.tile([128, 128], BF, tag="h2t")
                nc.vector.tensor_copy(h2t[:], tps[:])
                ps3 = psh.tile([128, 66], F32, tag="ps3", bufs=1)
                nc.tensor.matmul(ps3[:], lhsT=h2t[:], rhs=w2_sb[:],
                                 start=True, stop=True)
                h2row = hp.tile([128, 128], BF, tag="h2row")
                nc.vector.tensor_tensor(out=h2row[:, 0:64], in0=ps3[:, 0:64],
                                        in1=b2_sb[:], op=ALU.add)
                nc.vector.tensor_copy(h2row[:, 64:65], ps3[:, 64:65])
                nc.vector.tensor_copy(adst2[:, j:j + 1], ps3[:, 65:66])
                nc.sync.dma_start(out=h2_loc[j * 128:(j + 1) * 128, :],
                                  in_=h2row[:])

            def agg2(j):
                kj = int(K[j])
                cb = int(chunk_base[j])
                psum = psa.tile([128, 65], F32, tag="ps")
                z = zp.tile([128, KMAX], BF, tag="z2")
                az = zp.tile([128, KMAX], F32, tag="az2")
                for (b0, gl, in_hi) in batches_of(j):
                    k0 = cb + b0
                    gt = gp.tile([128, GBATCH * 128], BF, tag="g")
                    tab_ap = (h2_full[half:vpad, :] if in_hi
                              else h2_full[0:half, :])
                    nc.gpsimd.dma_gather(
                        gt[:, 0:gl * 128].rearrange("p (k c) -> p k c", c=128),
                        tab_ap,
                        idx_t[:, k0 * 8:(k0 + gl) * 8],
                        gl * 128, gl * 128, 128,
                        single_packet=False, queue_num=next_q(),
                    )
                    gv = gt[:, 0:gl * 128].rearrange("p (k c) -> p k c", c=128)
                    nc.scalar.activation(
                        az[:, b0:b0 + gl], gv[:, :, 64],
                        ACTF.Prelu, bias=adst2[:, j:j + 1], alpha=NEG_SLOPE)
                    nc.scalar.activation(z[:, b0:b0 + gl], az[:, b0:b0 + gl],
                                         ACTF.Exp)
                    nc.vector.tensor_tensor(
                        out=z[:, b0:b0 + gl], in0=z[:, b0:b0 + gl],
                        in1=msk_t[:, k0:k0 + gl], op=ALU.mult)
                    vt = vp.tile([128, GBATCH * 65], BF, tag="v2")
                    vv = vt[:, 0:gl * 65].rearrange("p (k c) -> p k c", c=65)
                    nc.vector.tensor_tensor(
                        out=vv[:, :, 0:64], in0=gv[:, :, 0:64],
                        in1=z[:, b0:b0 + gl].to_broadcast([128, gl, 64]),
                        op=ALU.mult)
                    nc.vector.tensor_copy(vv[:, :, 64:65],
                                          z[:, b0:b0 + gl].unsqueeze(2))
                    for k in range(gl):
                        nc.tensor.matmul(
                            psum[:], lhsT=ident[:],
                            rhs=vt[:, k * 65:(k + 1) * 65],
                            start=(b0 + k == 0), stop=(b0 + k == kj - 1))

                dsafe = wp.tile([128, 1], F32, tag="dsafe2")
                nc.vector.tensor_scalar_add(dsafe[:], psum[:, 64:65], 1e-30)
                rden = wp.tile([128, 1], F32, tag="rden2")
                nc.vector.reciprocal(rden[:], dsafe[:])
                ob = ep.tile([128, OUT], F32, tag="ob")
                nc.scalar.activation(ob[:], psum[:, 0:64], ACTF.Sigmoid,
                                     scale=rden[:, 0:1])
                nc.sync.dma_start(out=out_d[j * 128:(j + 1) * 128, :],
                                  in_=ob[:])

            for j in range(nb):
                agg1(j)
                if j + 1 in cuts:
                    ag_piece(h2_loc, h2_full, cuts.index(j + 1))
            for j in range(nb):
                agg2(j)

    nc.finalize()
    return nc


def kernel(x, edge_index, W1, att_src1, att_dst1, b1, W2, att_src2, att_dst2,
           b2):
    from concourse import bass_utils

    x = np.asarray(x, np.float32)
    W1 = np.asarray(W1, np.float32)
    W2 = np.asarray(W2, np.float32)
    att_src1 = np.asarray(att_src1, np.float32)
    att_dst1 = np.asarray(att_dst1, np.float32)
    att_src2 = np.asarray(att_src2, np.float32)
    att_dst2 = np.asarray(att_dst2, np.float32)
    b1 = np.asarray(b1, np.float32)
    b2 = np.asarray(b2, np.float32)
    n_nodes = x.shape[0]

    sch = _build_schedule(edge_index, n_nodes)
    vpad, pc = sch["vpad"], sch["pc"]

    W1r = W1.reshape(F_IN, HEADS, HID)
    w1_aug = np.zeros((F_IN, 130), np.float32)
    w1_aug[:, 0:HEADS * HID] = W1
    for h in range(HEADS):
        w1_aug[:, HEADS * HID + h] = W1r[:, h, :] @ att_dst1[h]
    w2_aug = np.zeros((HEADS * HID, 66), np.float32)
    w2_aug[:, 0:OUT] = W2
    w2_aug[:, OUT] = W2 @ att_src2[0]
    w2_aug[:, OUT + 1] = W2 @ att_dst2[0]
    b1_rep = np.broadcast_to(b1, (128, HEADS * HID)).copy()
    b2_rep = np.broadcast_to(b2, (128, OUT)).copy()
    asrc_rep = np.zeros((128, 128), np.float32)
    for h in range(HEADS):
        asrc_rep[:, h * HID:(h + 1) * HID] = att_src1[h]

    x_rho = np.zeros((vpad, F_IN), np.float32)
    x_rho[sch["row_of_node"]] = x

    key = (vpad, sch["nch"], tuple(sch["K"].tolist()),
           tuple(sch["K_lo"].tolist()), sch["cuts"])
    if key not in _cache:
        _cache[key] = _build_program(vpad, pc, sch["nb"], sch["half"],
                                     sch["K"], sch["K_lo"], sch["K_hi"],
                                     sch["nch"], sch["chunk_base"],
                                     sch["cuts"])
    nc = _cache[key]

    in_maps = []
    for c in range(NCORES):
        in_maps.append({
            "xt": np.ascontiguousarray(
                x_rho[c * pc:(c + 1) * pc].T).astype(BF16),
            "idx": sch["idx_wrapped"][c],
            "msk": sch["mask_stream"][c],
            "w1aug": w1_aug.astype(BF16),
            "w2aug": w2_aug.astype(BF16),
            "b1rep": b1_rep,
            "b2rep": b2_rep,
            "asrcrep": asrc_rep.astype(BF16),
        })
    res = bass_utils.run_bass_kernel_spmd(nc, in_maps,
                                          core_ids=list(range(NCORES)),
                                          trace=TRACE)
    kernel.last_exec_ns = res.exec_time_ns
    kernel.last_mean_ns = res.mean_exec_time_ns
    out_all = np.concatenate([res.results[c]["out"] for c in range(NCORES)], 0)
    return out_all[sch["row_of_node"][:n_nodes]]


# revision 32
# speedup vs baseline: 1.0325x; 1.0325x over previous
"""Two-layer GAT on 8 Trainium2 NeuronCores (Bass/Tile), bf16 gather path.

Host (numpy): append self-loops, degree-sort nodes (desc), pad node count to
VPAD (multiple of 8*128) and assign sorted nodes round-robin at 128-node
block granularity to the 8 cores (sorted-rank s -> block g=s//128,
lane=s%128 -> core c=g%8, local block j=g//8, table row = c*PC+j*128+lane).
Per block-rank j the chunk schedule is shared by all cores (SPMD: one
program, per-core tensor data).  Each dst node's edges occupy "slots"; a
chunk is slot k of all 128 lanes of a block.  Edge slots are split into a
lo-region (src table row < VPAD/2) and hi-region so every chunk's gather
indices fit in int16 after rebasing (dma_gather is int16-indexed).

The gather table holds ONLY h (+bias) as 128 bf16 = 256B rows (the dma
gather minimum).  Per-edge alpha_src is recomputed on-chip from the
gathered h row via a DVE multiply + free-dim reduce against a replicated
a_src constant; alpha_dst stays a per-lane [128,1] activation bias
(lane-aligned slots).  Softmax denominators come from a tensor_reduce over
the per-block z tile instead of extra V columns, so the segment-sum matmul
V is a pure 128-col bf16 tile accumulated into PSUM with an identity lhsT.
x, W, the table, and V are all bf16; PSUM accumulation fp32.
"""

import numpy as np
import ml_dtypes

BF16 = ml_dtypes.bfloat16

NCORES = 8
F_IN = 128
HID = 64
HEADS = 2
OUT = 64
NEG_SLOPE = 0.2

GBATCH = 64  # max chunks per dma_gather
KMAX = 64    # z-tile slot capacity (assert K.max() <= KMAX)
PIECES = 2   # AllGather pieces (table row space is piece-major)

TRACE = False
_cache = {}


def _build_schedule(edge_index, n_nodes):
    ei = np.asarray(edge_index).astype(np.int64)
    src = np.concatenate([ei[0], np.arange(n_nodes, dtype=np.int64)])
    dst = np.concatenate([ei[1], np.arange(n_nodes, dtype=np.int64)])
    deg = np.bincount(dst, minlength=n_nodes)

    stripe = NCORES * 128
    vpad = ((n_nodes + stripe - 1) // stripe) * stripe
    pc = vpad // NCORES
    nb = pc // 128
    half = vpad // 2
    assert half <= 32768

    # AllGather piece boundaries (in blocks); the gather-table row space is
    # piece-major (piece, core, block-within-piece, lane) so each piece's
    # collective output is one contiguous slice of h_full.
    cuts = sorted(set(min(nb, ((nb * (i + 1)) + PIECES - 1) // PIECES)
                      for i in range(PIECES)))
    starts = [0] + cuts[:-1]
    piece_of_j = np.zeros(nb, np.int64)
    tbase_of_j = np.zeros(nb, np.int64)
    cstr_of_j = np.zeros(nb, np.int64)
    joff_of_j = np.zeros(nb, np.int64)
    base = 0
    for p, (j0, j1) in enumerate(zip(starts, cuts)):
        bp = j1 - j0
        for j in range(j0, j1):
            piece_of_j[j] = p
            tbase_of_j[j] = base
            cstr_of_j[j] = bp * 128
            joff_of_j[j] = (j - j0) * 128
        base += NCORES * bp * 128
    assert base == vpad

    degp = np.zeros(vpad, np.int64)
    degp[:n_nodes] = deg
    order = np.argsort(-degp, kind="stable")
    rank = np.empty(vpad, np.int64)
    rank[order] = np.arange(vpad)

    s = np.arange(vpad)
    g = s // 128
    lane_r = s % 128
    c_r = g % NCORES
    j_r = g // NCORES
    local_of_rank = c_r * pc + j_r * 128 + lane_r
    trow_of_rank = (tbase_of_j[j_r] + c_r * cstr_of_j[j_r]
                    + joff_of_j[j_r] + lane_r)
    nrank = rank[:n_nodes]
    row_of_node = local_of_rank[nrank]       # for x placement / output
    trow_of_node = trow_of_rank[nrank]       # for gather indices
    c_of_node = c_r[nrank]
    j_of_node = j_r[nrank]
    lane_of_node = lane_r[nrank]

    e_srcrow = trow_of_node[src]
    is_hi = e_srcrow >= half
    e_c = c_of_node[dst]
    e_j = j_of_node[dst]
    e_lane = lane_of_node[dst]
    e_dkey = (e_c * pc // 128 + e_j) * 128 + e_lane  # dense (c, j, lane) id

    # per-(dst, lo/hi) counts
    nkeys = NCORES * nb * 128
    lo_cnt = np.bincount(e_dkey[~is_hi], minlength=nkeys)
    hi_cnt = np.bincount(e_dkey[is_hi], minlength=nkeys)

    # per-block-rank shared chunk counts: max over the 8 cores' j-th blocks
    jj = (np.arange(nkeys) // 128) % nb
    K_lo = np.zeros(nb, np.int64)
    K_hi = np.zeros(nb, np.int64)
    np.maximum.at(K_lo, jj, lo_cnt)
    np.maximum.at(K_hi, jj, hi_cnt)
    K = K_lo + K_hi
    bump = K == 0
    K_lo[bump] += 1
    K[bump] += 1
    nch = int(K.sum())
    chunk_base = np.concatenate([[0], np.cumsum(K)])[:-1]

    # slot assignment: edges of a dst grouped, lo first then hi
    key = e_dkey * 2 + is_hi
    ord_e = np.argsort(key, kind="stable")
    ds = e_dkey[ord_e]
    hs = is_hi[ord_e]
    first = np.r_[True, ds[1:] != ds[:-1]]
    grp_start = np.flatnonzero(first)
    grp_id = np.cumsum(first) - 1
    slot = np.arange(ds.shape[0]) - grp_start[grp_id]
    c = ds // (nb * 128)
    j = (ds // 128) % nb
    ln = ds % 128
    pos = chunk_base[j] + np.where(hs, K_lo[j] + slot - lo_cnt[ds], slot)
    assert (pos >= chunk_base[j]).all() and (pos < chunk_base[j] + K[j]).all()

    idx_val = np.where(hs, e_srcrow[ord_e] - half, e_srcrow[ord_e])
    idx_stream = np.zeros((NCORES, 128, nch), np.int16)
    mask_stream = np.zeros((NCORES, 128, nch), BF16)
    idx_stream[c, ln, pos] = idx_val.astype(np.int16)
    mask_stream[c, ln, pos] = 1.0

    # wrapped int16 layout for dma_gather: chunk k -> columns 8k:8k+8 of
    # [128, 8*nch]; within a chunk the 128 lane-indices are wrapped as
    # flat[i] -> [i % 16, i // 16] and replicated over the 8 16-partition
    # groups.
    iw = idx_stream.transpose(0, 2, 1).reshape(NCORES, nch, 8, 16)
    iw = iw.transpose(0, 3, 1, 2).reshape(NCORES, 16, nch * 8)
    idx_wrapped = np.tile(iw, (1, 8, 1))

    return dict(vpad=vpad, pc=pc, nb=nb, half=half, K=K, K_lo=K_lo, K_hi=K_hi,
                nch=nch, chunk_base=chunk_base, row_of_node=row_of_node,
                cuts=tuple(cuts),
                idx_wrapped=np.ascontiguousarray(idx_wrapped),
                mask_stream=np.ascontiguousarray(mask_stream))


def _build_program(vpad, pc, nb, half, K, K_lo, K_hi, nch, chunk_base, cuts):
    import concourse.bacc as bacc
    import concourse.bass as bass
    import concourse.mybir as mybir
    import concourse.tile as tile
    from concourse.masks import make_identity

    F32 = mybir.dt.float32
    BF = mybir.dt.bfloat16
    I16 = mybir.dt.int16
    ACTF = mybir.ActivationFunctionType
    ALU = mybir.AluOpType
    AXL = mybir.AxisListType

    assert int(K.max()) <= KMAX

    nc = bacc.Bacc("TRN2", target_bir_lowering=False, debug=False,
                   num_devices=NCORES, num_swdge_queues=4)
    qn = [0]

    def next_q():
        q = qn[0]
        qn[0] = (q + 1) % 4
        return q

    xt_d = nc.dram_tensor("xt", [128, pc], BF, kind="ExternalInput")
    idx_d = nc.dram_tensor("idx", [128, nch * 8], I16, kind="ExternalInput")
    msk_d = nc.dram_tensor("msk", [128, nch], BF, kind="ExternalInput")
    w1_d = nc.dram_tensor("w1aug", [128, 130], BF, kind="ExternalInput")
    w2_d = nc.dram_tensor("w2aug", [128, 66], BF, kind="ExternalInput")
    b1_d = nc.dram_tensor("b1rep", [128, 128], F32, kind="ExternalInput")
    b2_d = nc.dram_tensor("b2rep", [128, 64], F32, kind="ExternalInput")
    as_d = nc.dram_tensor("asrcrep", [128, 128], BF, kind="ExternalInput")
    out_d = nc.dram_tensor("out", [pc, OUT], F32, kind="ExternalOutput")

    with tile.TileContext(nc) as tc:
        with (
            tc.tile_pool(name="const", bufs=1) as cp,
            tc.tile_pool(name="dram", bufs=1, space="DRAM") as dp,
            tc.tile_pool(name="hrow", bufs=3) as hp,
            tc.tile_pool(name="psh", bufs=2, space="PSUM") as psh,
            tc.tile_pool(name="g", bufs=4) as gp,
            tc.tile_pool(name="sc", bufs=2) as sp,
            tc.tile_pool(name="v", bufs=2) as vp,
            tc.tile_pool(name="z", bufs=2) as zp,
            tc.tile_pool(name="wz", bufs=4) as wp,
            tc.tile_pool(name="psa", bufs=2, space="PSUM") as psa,
            tc.tile_pool(name="pst", bufs=1, space="PSUM") as pst,
            tc.tile_pool(name="epi", bufs=3) as ep,
        ):
            ident = cp.tile([128, 128], BF)
            make_identity(nc, ident[:])
            w1_sb = cp.tile([128, 130], BF)
            w2_sb = cp.tile([128, 66], BF)
            b1_sb = cp.tile([128, 128], F32)
            b2_sb = cp.tile([128, 64], F32)
            as_sb = cp.tile([128, 128], BF)
            idx_t = cp.tile([128, nch * 8], I16)
            msk_t = cp.tile([128, nch], BF)
            xt_all = cp.tile([128, pc], BF)
            adst1 = cp.tile([128, 2 * nb], F32)
            adst2 = cp.tile([128, nb], F32)
            for t, d in ((w1_sb, w1_d), (w2_sb, w2_d), (b1_sb, b1_d),
                         (b2_sb, b2_d), (as_sb, as_d), (idx_t, idx_d),
                         (msk_t, msk_d), (xt_all, xt_d)):
                nc.sync.dma_start(out=t[:], in_=d[:])

            h_loc = dp.tile([pc, 128], BF)
            h2_loc = dp.tile([pc, 128], BF)
            h_full = dp.tile([vpad, 128], BF)
            h2_full = dp.tile([vpad, 128], BF)

            # piece p = blocks [starts[p], cuts[p]); its table rows are the
            # contiguous [tb, tb + 8*bp*128) slice (piece-major row space)
            starts = [0] + list(cuts[:-1])
            tbases = []
            tb = 0
            for j0, j1 in zip(starts, cuts):
                tbases.append(tb)
                tb += NCORES * (j1 - j0) * 128
            assert tb == vpad

            def ag_piece(loc, full, p):
                j0, j1 = starts[p], cuts[p]
                nc.gpsimd.collective_compute(
                    "AllGather", mybir.AluOpType.bypass,
                    replica_groups=[list(range(NCORES))],
                    ins=[loc[j0 * 128:j1 * 128, :]],
                    outs=[full[tbases[p]:tbases[p]
                               + NCORES * (j1 - j0) * 128, :]],
                )

            # ---- Phase 1: L1 h-phase (AllGather in overlapped pieces) ----
            for j in range(nb):
                ps = psh.tile([128, 130], F32, tag="psh")
                nc.tensor.matmul(ps[:], lhsT=xt_all[:, j * 128:(j + 1) * 128],
                                 rhs=w1_sb[:], start=True, stop=True)
                hrow = hp.tile([128, 128], BF, tag="hrow")
                nc.vector.tensor_tensor(out=hrow[:], in0=ps[:, 0:128],
                                        in1=b1_sb[:], op=ALU.add)
                nc.scalar.copy(adst1[:, j * 2:(j + 1) * 2], ps[:, 128:130])
                nc.sync.dma_start(out=h_loc[j * 128:(j + 1) * 128, :],
                                  in_=hrow[:])
                if j + 1 in cuts:
                    ag_piece(h_loc, h_full, cuts.index(j + 1))

            def batches_of(j):
                out = []
                done = 0
                for rl in (int(K_lo[j]), int(K_hi[j])):
                    r0 = done
                    while done < r0 + rl:
                        gl = min(GBATCH, r0 + rl - done)
                        out.append((done, gl, done >= int(K_lo[j])))
                        done += gl
                return out

            def agg1(j):
                kj = int(K[j])
                cb = int(chunk_base[j])
                psum = psa.tile([128, 130], F32, tag="ps")
                az = zp.tile([128, 2, KMAX], F32, tag="az")
                for (b0, gl, in_hi) in batches_of(j):
                    k0 = cb + b0
                    gt = gp.tile([128, GBATCH * 128], BF, tag="g")
                    tab_ap = h_full[half:vpad, :] if in_hi else h_full[0:half, :]
                    nc.gpsimd.dma_gather(
                        gt[:, 0:gl * 128].rearrange("p (k c) -> p k c", c=128),
                        tab_ap,
                        idx_t[:, k0 * 8:(k0 + gl) * 8],
                        gl * 128, gl * 128, 128,
                        single_packet=False, queue_num=next_q(),
                    )
                    gv = gt[:, 0:gl * 128].rearrange("p (k c) -> p k c", c=128)
                    # alpha_src per edge: dot(h_row, a_src) over each head's
                    # 64 cols, both heads in one mult + one reduce
                    scr = sp.tile([128, GBATCH * 128], BF, tag="s", bufs=1)
                    sv = scr[:, 0:gl * 128].rearrange(
                        "p (k h c) -> p k h c", h=2, c=HID)
                    nc.vector.tensor_tensor(
                        out=sv, in0=gv.rearrange("p k (h c) -> p k h c", c=HID),
                        in1=as_sb[:].rearrange("p (h c) -> p h c", c=HID)
                            .unsqueeze(1).broadcast_to([128, gl, 2, HID]),
                        op=ALU.mult)
                    azr = az[:, :, :].rearrange("p h k -> p k h")
                    nc.vector.tensor_reduce(
                        out=azr[:, b0:b0 + gl, :], in_=sv,
                        axis=AXL.X, op=ALU.add)
                    vt = vp.tile([128, GBATCH * 130], BF, tag="v")
                    vv = vt[:, 0:gl * 130].rearrange("p (k c) -> p k c", c=130)
                    for h in range(HEADS):
                        nc.scalar.activation(
                            az[:, h, b0:b0 + gl], az[:, h, b0:b0 + gl],
                            ACTF.Prelu, bias=adst1[:, 2 * j + h:2 * j + h + 1],
                            alpha=NEG_SLOPE)
                        # w = exp(.) written straight into V's den column
                        nc.scalar.activation(
                            vv[:, :, 128 + h], az[:, h, b0:b0 + gl],
                            ACTF.Exp)
                    nc.vector.tensor_tensor(
                        out=vv[:, :, 128:130], in0=vv[:, :, 128:130],
                        in1=msk_t[:, k0:k0 + gl].unsqueeze(2)
                            .broadcast_to([128, gl, 2]),
                        op=ALU.mult)
                    nc.vector.tensor_tensor(
                        out=vv[:, :, 0:128].rearrange(
                            "p k (h c) -> p k h c", c=HID),
                        in0=gv.rearrange("p k (h c) -> p k h c", c=HID),
                        in1=vv[:, :, 128:130].unsqueeze(3)
                            .broadcast_to([128, gl, 2, HID]),
                        op=ALU.mult)
                    for k in range(gl):
                        nc.tensor.matmul(
                            psum[:], lhsT=ident[:],
                            rhs=vt[:, k * 130:(k + 1) * 130],
                            start=(b0 + k == 0), stop=(b0 + k == kj - 1))

                # +1e-30 keeps all-padding lanes finite (0 -> 1e30 -> 0*1e30=0)
                dsafe = wp.tile([128, 2], F32, tag="dsafe")
                nc.vector.tensor_scalar_add(dsafe[:], psum[:, 128:130], 1e-30)
                rden = wp.tile([128, 2], F32, tag="rden")
                nc.vector.reciprocal(rden[:], dsafe[:])

                h2pre = ep.tile([128, 128], BF, tag="h2pre")
                for h in range(HEADS):
                    nc.vector.tensor_scalar(
                        out=h2pre[:, h * HID:(h + 1) * HID],
                        in0=psum[:, h * HID:(h + 1) * HID],
                        scalar1=rden[:, h:h + 1], scalar2=0.0,
                        op0=ALU.mult, op1=ALU.max)
                tps = pst.tile([128, 128], BF, tag="tps")
                nc.tensor.transpose(out=tps[:], in_=h2pre[:], identity=ident[:])
                h2t = ep.tile([128, 128], BF, tag="h2t")
                nc.scalar.copy(h2t[:], tps[:])
                ps3 = psh.tile([128, 66], F32, tag="ps3", bufs=1)
                nc.tensor.matmul(ps3[:], lhsT=h2t[:], rhs=w2_sb[:],
                                 start=True, stop=True)
                h2row = hp.tile([128, 128], BF, tag="h2row")
                nc.vector.tensor_tensor(out=h2row[:, 0:64], in0=ps3[:, 0:64],
                                        in1=b2_sb[:], op=ALU.add)
                nc.scalar.copy(h2row[:, 64:65], ps3[:, 64:65])
                nc.scalar.copy(adst2[:, j:j + 1], ps3[:, 65:66])
                nc.sync.dma_start(out=h2_loc[j * 128:(j + 1) * 128, :],
                                  in_=h2row[:])

            def agg2(j):
                kj = int(K[j])
                cb = int(chunk_base[j])
                psum = psa.tile([128, 65], F32, tag="ps")
                az = zp.tile([128, KMAX], F32, tag="az2")
                for (b0, gl, in_hi) in batches_of(j):
                    k0 = cb + b0
                    gt = gp.tile([128, GBATCH * 128], BF, tag="g")
                    tab_ap = (h2_full[half:vpad, :] if in_hi
                              else h2_full[0:half, :])
                    nc.gpsimd.dma_gather(
                        gt[:, 0:gl * 128].rearrange("p (k c) -> p k c", c=128),
                        tab_ap,
                        idx_t[:, k0 * 8:(k0 + gl) * 8],
                        gl * 128, gl * 128, 128,
                        single_packet=False, queue_num=next_q(),
                    )
                    gv = gt[:, 0:gl * 128].rearrange("p (k c) -> p k c", c=128)
                    vt = vp.tile([128, GBATCH * 65], BF, tag="v2")
                    vv = vt[:, 0:gl * 65].rearrange("p (k c) -> p k c", c=65)
                    nc.scalar.activation(
                        az[:, b0:b0 + gl], gv[:, :, 64],
                        ACTF.Prelu, bias=adst2[:, j:j + 1], alpha=NEG_SLOPE)
                    nc.scalar.activation(vv[:, :, 64], az[:, b0:b0 + gl],
                                         ACTF.Exp)
                    nc.vector.tensor_tensor(
                        out=vv[:, :, 64:65], in0=vv[:, :, 64:65],
                        in1=msk_t[:, k0:k0 + gl].unsqueeze(2),
                        op=ALU.mult)
                    nc.vector.tensor_tensor(
                        out=vv[:, :, 0:64], in0=gv[:, :, 0:64],
                        in1=vv[:, :, 64:65].broadcast_to([128, gl, 64]),
                        op=ALU.mult)
                    for k in range(gl):
                        nc.tensor.matmul(
                            psum[:], lhsT=ident[:],
                            rhs=vt[:, k * 65:(k + 1) * 65],
                            start=(b0 + k == 0), stop=(b0 + k == kj - 1))

                dsafe = wp.tile([128, 1], F32, tag="dsafe2")
                nc.vector.tensor_scalar_add(dsafe[:], psum[:, 64:65], 1e-30)
                rden = wp.tile([128, 1], F32, tag="rden2")
                nc.vector.reciprocal(rden[:], dsafe[:])
                ob = ep.tile([128, OUT], F32, tag="ob")
                nc.scalar.activation(ob[:], psum[:, 0:64], ACTF.Sigmoid,
                                     scale=rden[:, 0:1])
                nc.sync.dma_start(out=out_d[j * 128:(j + 1) * 128, :],
                                  in_=ob[:])

            for j in range(nb):
                agg1(j)
                if j + 1 in cuts:
                    ag_piece(h2_loc, h2_full, cuts.index(j + 1))
            for j in range(nb):
                agg2(j)

    nc.finalize()
    return nc


def kernel(x, edge_index, W1, att_src1, att_dst1, b1, W2, att_src2, att_dst2,
           b2):
    from concourse import bass_utils

    x = np.asarray(x, np.float32)
    W1 = np.asarray(W1, np.float32)
    W2 = np.asarray(W2, np.float32)
    att_src1 = np.asarray(att_src1, np.float32)
    att_dst1 = np.asarray(att_dst1, np.float32)
    att_src2 = np.asarray(att_src2, np.float32)
    att_dst2 = np.asarray(att_dst2, np.float32)
    b1 = np.asarray(b1, np.float32)
    b2 = np.asarray(b2, np.float32)
    n_nodes = x.shape[0]

    sch = _build_schedule(edge_index, n_nodes)
    vpad, pc = sch["vpad"], sch["pc"]

    W1r = W1.reshape(F_IN, HEADS, HID)
    w1_aug = np.zeros((F_IN, 130), np.float32)
    w1_aug[:, 0:HEADS * HID] = W1
    for h in range(HEADS):
        w1_aug[:, HEADS * HID + h] = W1r[:, h, :] @ att_dst1[h]
    w2_aug = np.zeros((HEADS * HID, 66), np.float32)
    w2_aug[:, 0:OUT] = W2
    w2_aug[:, OUT] = W2 @ att_src2[0]
    w2_aug[:, OUT + 1] = W2 @ att_dst2[0]
    b1_rep = np.broadcast_to(b1, (128, HEADS * HID)).copy()
    b2_rep = np.broadcast_to(b2, (128, OUT)).copy()
    asrc_rep = np.zeros((128, 128), np.float32)
    for h in range(HEADS):
        asrc_rep[:, h * HID:(h + 1) * HID] = att_src1[h]

    x_rho = np.zeros((vpad, F_IN), np.float32)
    x_rho[sch["row_of_node"]] = x

    key = (vpad, sch["nch"], tuple(sch["K"].tolist()),
           tuple(sch["K_lo"].tolist()), sch["cuts"])
    if key not in _cache:
        _cache[key] = _build_program(vpad, pc, sch["nb"], sch["half"],
                                     sch["K"], sch["K_lo"], sch["K_hi"],
                                     sch["nch"], sch["chunk_base"],
                                     sch["cuts"])
    nc = _cache[key]

    in_maps = []
    for c in range(NCORES):
        in_maps.append({
            "xt": np.ascontiguousarray(
                x_rho[c * pc:(c + 1) * pc].T).astype(BF16),
            "idx": sch["idx_wrapped"][c],
            "msk": sch["mask_stream"][c],
            "w1aug": w1_aug.astype(BF16),
            "w2aug": w2_aug.astype(BF16),
            "b1rep": b1_rep,
            "b2rep": b2_rep,
            "asrcrep": asrc_rep.astype(BF16),
        })
    res = bass_utils.run_bass_kernel_spmd(nc, in_maps,
                                          core_ids=list(range(NCORES)),
                                          trace=TRACE)
    kernel.last_exec_ns = res.exec_time_ns
    kernel.last_mean_ns = res.mean_exec_time_ns
    out_all = np.concatenate([res.results[c]["out"] for c in range(NCORES)], 0)
    return out_all[sch["row_of_node"][:n_nodes]]
